# revision 36
# baseline (speedup 1.0000x reference)
"""BrushStroke splat kernel for 8 trn2 NeuronCores (v3).

out[b,c,y,x] = mean_n sum_{p,q} Fy[b,n,y,p] Fx[b,n,x,q] patches[b,n,c,p,q]
with Fx/Fy separable Gaussian filter banks (sigma=0.1) normalized over a
padded spatial axis.

Per core (2 batches of 64 strokes, batch-parallel across cores):
 - E rows E[r,t] = exp(-(t - c_r)^2 / (2 sigma^2)), t in [0,319), one per
   (stroke, batch) on partition r = 32j + 2g + b, stored bf16 as one
   [128, 638] x||y tile and bounced to DRAM.
 - One-time prologue computes all 64x32 window-sum normalizers per side
   (window = T - prefix - suffix via tensor_tensor_scan), reciprocal,
   then a DVE 32x32 block transpose -> per-partition scales IVX/IVY.
 - Per group one bf16 DMA gather [128,512] provides both Fx and Fy
   shifted windows; Fx normalizer is one tensor_scalar_mul; Fy windows
   feed MM2 raw (its normalizer is folded into the MM1 PSUM drain).
 - MM1 per (g,c): full-array bf16 matmul with block-diagonal lhsT
   (zeros embedded host-side, bf16, DMA'd in column chunks).
 - MM2 per (b,g): 2 LDW x 4 chained matmuls into 4 PSUM banks
   (yt0/yt1 x {512-wide c0c1, 256-wide c2}).
 - f32 warmup matmuls during the prologue keep the PE HAM warm.
"""
import sys, types
import numpy as np

IMAGE = 256
PAD = 16
EPS = 1e-7
SIGMA2 = 2.0 * 0.1 ** 2
B, N, C, PH, PW = 16, 64, 3, 32, 32
NCORES = 8
BLOC = B // NCORES          # 2 batches per core
NG = N // 4                 # 16 groups of 4 strokes
ET = IMAGE + 2 * PAD + PW - 1   # 319: E row length
WLEN = IMAGE + 2 * PAD          # 288: padded-axis (normalizer window) length


def _install_patches():
    if 'antenv.axon_hooks' not in sys.modules:
        mod = types.ModuleType('antenv.axon_hooks')
        mod._hook = None
        mod.set_axon_ntff_profile_hook = lambda h: setattr(mod, '_hook', h)
        mod.get_axon_ntff_profile_hook = lambda: mod._hook
        sys.modules['antenv.axon_hooks'] = mod
        try:
            from trn_agent_boot.trn_boot import _ntff_profile_via_ctypes
            hook = _ntff_profile_via_ctypes('/opt/axon/libaxon_pjrt.so')
            if hook is not None:
                mod.set_axon_ntff_profile_hook(hook)
        except Exception:
            pass

    import concourse.tile as tile
    import concourse.bass_utils as bass_utils
    from concourse.vector_clock import ScopedClock

    bass_utils.upload_artifacts = lambda tmpdir: 'local://' + tmpdir

    if getattr(tile.TileContext._drain_and_barrier, '_patched', False):
        return

    def _drain_and_barrier(self, tick_clock, wait_clock):
        nc = self.nc
        drain_inst = nc.sync.drain()
        wait_clock.add_sem_waits(
            drain_inst.ins, ScopedClock({None: tick_clock.global_clock}))
        si = drain_inst.ins.sync_info
        waits = list(si.on_wait or [])
        si.on_wait = []
        for w in waits:
            nop = nc.sync.nop()
            nop.ins.sync_info = type(si)(on_wait=[w], on_update=[])
        nc.all_engine_barrier()
        popped = nc._tile_sem_poison_stack.pop()
        assert popped is self._sem_poison
        nc.clear_and_free_semaphores(list(self.sems.allocated().values()))
        nc.all_engine_barrier()

    _drain_and_barrier._patched = True
    tile.TileContext._drain_and_barrier = _drain_and_barrier


def _split_multi_waits(nc):
    """This walrus accepts at most one sync wait per instruction; hoist
    extras onto same-engine NoOps inserted just before."""
    import bass_rust
    n_new = [0]

    def fresh_nop(engine, wait, si_type):
        n_new[0] += 1
        nop = bass_rust.InstNoOp(name=f'I-waitsplit-{n_new[0]}', ins=[], outs=[])
        nop.engine = engine
        nop.sync_info = si_type(on_wait=[wait], on_update=[])
        return nop

    for fn in nc.m.functions:
        for blk in fn.blocks:
            insts = blk.instructions
            i = 0
            while i < len(insts):
                inst = insts[i]
                si = inst.sync_info
                if si is not None and si.on_wait and len(si.on_wait) > 1:
                    waits = list(si.on_wait)
                    si.on_wait = [waits[-1]]
                    for k, w in enumerate(waits[:-1]):
                        insts.insert(i + k, fresh_nop(inst.engine, w, type(si)))
                    i += len(waits) - 1
                i += 1


_PROGRAM = None


def _build_program():
    global _PROGRAM
    if _PROGRAM is not None:
        return _PROGRAM
    _install_patches()
    import concourse.bass as bass
    import concourse.tile as tile
    from concourse import mybir
    from bass_rust import AP

    f32 = mybir.dt.float32
    bf16 = mybir.dt.bfloat16
    AF = mybir.ActivationFunctionType
    AX = mybir.AxisListType
    ALU = mybir.AluOpType

    nc = bass.Bass('TRN2', target_bir_lowering=False, debug=False,
                   num_devices=NCORES)
    # per-core inputs:
    #  g2w  [5,133]: cols 0:128 rows 0-3 brush coords (x_b0,x_b1,y_b0,y_b1;
    #        col r = stroke nu(r)), row 4 = batch-select r%2;
    #        cols 128:133 = 5x5 identity
    #  pt2  [2,128,6144] bf16: full block-diagonal lhsT (zeros embedded)
    g2w_in = nc.declare_dram_parameter('g2w', [5, 133], f32, isOutput=False)
    pt_in = nc.declare_dram_parameter('pt2', [BLOC, 128, 128 * C * NG], bf16,
                                      isOutput=False)
    y_out = nc.declare_dram_parameter('y_out', [BLOC, C, IMAGE, IMAGE], f32,
                                      isOutput=True)

    E_dram = nc.dram_tensor('E_dram', [128, 2 * ET], bf16)

    with tile.TileContext(nc) as tc:
        with tc.tile_pool(name='glob', bufs=1) as gp, \
             tc.tile_pool(name='fxyp', bufs=2) as fxyp, \
             tc.tile_pool(name='fxnp', bufs=4) as fxnp, \
             tc.tile_pool(name='tgp', bufs=1) as tgp, \
             tc.tile_pool(name='obp', bufs=2) as obp, \
             tc.tile_pool(name='mm1ps', bufs=2, space='PSUM') as mm1ps, \
             tc.tile_pool(name='mm2ps', bufs=1, space='PSUM') as mm2ps:
            # ---- input DMAs ----
            g2w = gp.tile([5, 133], f32)
            nc.sync.dma_start(g2w[:], g2w_in[:])
            g2 = g2w[0:4, 0:128]
            idt = g2w[:, 128:133]

            psall = []
            for b in range(BLOC):
                ps = gp.tile([128, 128 * C * NG], bf16, name=f'psall{b}')
                psall.append(ps)
            CHUNK = 1536
            for ch in range(4):          # batch-0 fills, early, on sync
                nc.sync.dma_start(psall[0][:, CHUNK * ch:CHUNK * (ch + 1)],
                                  pt_in[0, :, CHUNK * ch:CHUNK * (ch + 1)])

            # ---- iotas (gpsimd) ----
            it = gp.tile([128, ET], f32)
            nc.gpsimd.iota(it[:], pattern=[[1, ET]], base=0,
                           channel_multiplier=0,
                           allow_small_or_imprecise_dtypes=True)
            # t^2 early (also the warmup matmul operand)
            t2 = gp.tile([128, ET], f32)
            nc.vector.tensor_mul(t2[:], it[:], it[:])

            # ---- brush normalization -> bias vectors ----
            g25 = g2w[0:5, 0:128]
            mn = gp.tile([5, 1], f32)
            mx = gp.tile([5, 1], f32)
            nc.vector.tensor_reduce(mn[:], g25, axis=AX.X, op=ALU.min)
            nc.vector.reduce_max(mx[:], g25, axis=AX.X)
            rng = gp.tile([5, 1], f32)
            nc.vector.tensor_scalar(rng[:], mx[:], mn[:], EPS,
                                    ALU.subtract, ALU.add)
            inv = gp.tile([5, 1], f32)
            nc.vector.reciprocal(inv[:], rng[:])
            gn = gp.tile([5, 128], f32)
            nc.vector.tensor_scalar(gn[:], g25, mn[:], inv[:],
                                    ALU.subtract, ALU.mult)

            tp_ps = mm2ps.tile([128, 5], f32, tag='A0')
            nc.tensor.transpose(tp_ps[:], gn[:], idt)
            tp = gp.tile([128, 5], f32)
            nc.scalar.copy(tp[:], tp_ps[:])
            bs = tp[:, 4:5]

            # bias_x[r] = -(256*gxn[b(r),nu(r)] + CX)
            CXC = PW / 2 - 0.5 + PAD      # 31.5
            CYC = PW / 2 - 0.4 + PAD      # 31.6
            bias = {}
            for nmo, (c0, c1, CC) in {'x': (0, 1, CXC),
                                      'y': (2, 3, CYC)}.items():
                d01 = gp.tile([128, 1], f32, name=f'd{nmo}')
                v = gp.tile([128, 1], f32, name=f'v{nmo}')
                bi = gp.tile([128, 1], f32, name=f'bias{nmo}')
                nc.vector.tensor_sub(d01[:], tp[:, c1:c1 + 1], tp[:, c0:c0 + 1])
                nc.vector.scalar_tensor_tensor(v[:], d01[:], bs,
                                               tp[:, c0:c0 + 1],
                                               ALU.mult, ALU.add)
                nc.vector.tensor_scalar(bi[:], v[:], -float(IMAGE), CC,
                                        ALU.mult, ALU.subtract)
                bias[nmo] = bi

            # ---- E rows: exp(-(t+b)^2/S2); (t+b) first to avoid fp32
            # cancellation in the expanded square
            E = gp.tile([128, 2 * ET], bf16)
            for nmo, off in (('x', 0), ('y', ET)):
                dd = gp.tile([128, ET], f32, name=f'dd{nmo}')
                nc.vector.tensor_scalar_add(dd[:], it[:], bias[nmo][:])
                sq = gp.tile([128, ET], f32, name=f'sq{nmo}')
                nc.vector.tensor_mul(sq[:], dd[:], dd[:])
                nc.scalar.activation(E[:, off:off + ET], sq[:], AF.Exp,
                                     bias=0.0, scale=-1.0 / SIGMA2)
            nc.sync.dma_start(E_dram[:], E[:])

            # ---- warmup matmuls (keep PE busy through the prologue) ----
            for w in range(8):
                wps = mm1ps.tile([128, 512], f32, name='wps', tag='p01')
                nc.tensor.matmul(wps[:, 0:IMAGE], t2[:, 0:128], t2[:, 0:IMAGE],
                                 start=True, stop=True)

            # ---- normalizers: D[r,q'] = T - prefix - suffix; IV = 1/D ----
            def normalizer(off, name):
                Es = E[:, off:off + ET]
                T = gp.tile([128, 1], f32, name=f'T{name}')
                nc.vector.reduce_sum(T[:], Es, axis=AX.X)
                P1 = gp.tile([128, PW], f32, name=f'P1{name}')
                nc.vector.tensor_tensor_scan(P1[:], Es[:, 0:PW], Es[:, 0:PW],
                                             0.0, ALU.add, ALU.bypass)
                P2 = gp.tile([128, PW - 1], f32, name=f'P2{name}')
                nc.vector.tensor_tensor_scan(P2[:], Es[:, WLEN:ET],
                                             Es[:, WLEN:ET],
                                             0.0, ALU.add, ALU.bypass)
                T2 = gp.tile([128, 1], f32, name=f'T2{name}')
                nc.vector.scalar_tensor_tensor(T2[:], T[:], EPS,
                                               P2[:, PW - 2:PW - 1],
                                               ALU.add, ALU.subtract)
                Q = gp.tile([128, PW - 1], f32, name=f'Q{name}')
                nc.vector.tensor_sub(Q[:], P2[:], P1[:, 0:PW - 1])
                D = gp.tile([128, PW], f32, name=f'D{name}')
                nc.vector.tensor_copy(D[:, 0:1], T2[:])
                nc.vector.tensor_scalar_add(D[:, 1:PW], Q[:], T2[:])
                REC = gp.tile([128, PW], f32, name=f'REC{name}')
                nc.vector.reciprocal(REC[:], D[:])
                IV = gp.tile([128, PW], f32, name=f'IV{name}')
                nc.vector.transpose(IV[:], REC[:])
                return IV

            IVX = normalizer(0, 'x')
            IVY = normalizer(ET, 'y')

            # batch-1 lhsT fills (scalar queue, after the E activations)
            for ch in range(4):
                nc.scalar.dma_start(psall[1][:, CHUNK * ch:CHUNK * (ch + 1)],
                                    pt_in[1, :, CHUNK * ch:CHUNK * (ch + 1)])

            # ---- main loops ----
            for b in range(BLOC):
                fxy_tiles = {}
                tg_tiles = {}
                accs = {}
                for yt in range(2):
                    accs[(yt, 'A')] = mm2ps.tile([128, 512], f32,
                                                 name=f'A{yt}', tag=f'A{yt}')
                    accs[(yt, 'B')] = mm2ps.tile([128, IMAGE], f32,
                                                 name=f'B{yt}', tag=f'B{yt}')

                def mm2_step(g):
                    st, sp = (g == 0), (g == NG - 1)
                    tgs = tg_tiles[g]
                    for yt in range(2):
                        l = fxy_tiles[g][:, 128 * yt:128 * yt + 128]
                        nc.tensor.matmul(accs[(yt, 'A')][:], l, tgs[:, 0:512],
                                         start=st, stop=sp)
                        nc.tensor.matmul(accs[(yt, 'B')][:], l,
                                         tgs[:, 512:768], start=st, stop=sp)

                for g in range(NG):
                    m = 2 * g + b
                    fxg = fxnp.tile([128, IMAGE], bf16, name='fxg', tag='fxg')
                    nc.gpsimd.dma_start(
                        fxg[:], AP(E_dram, m * 2 * ET + PAD,
                                   [[2 * ET * 32, 4], [1, PW], [1, IMAGE]]))
                    fyg = fxyp.tile([128, IMAGE], bf16, name=f'fy{g}',
                                    tag=f'fy{g}')
                    feng = nc.sync if g % 2 == 0 else nc.scalar
                    feng.dma_start(
                        fyg[:], AP(E_dram, m * 2 * ET + ET + PAD,
                                   [[2 * ET * 32, 4], [1, PW], [1, IMAGE]]))
                    fxy_tiles[g] = fyg
                    fxn = fxnp.tile([128, IMAGE], bf16, name='fxn', tag='fxn')
                    nc.vector.tensor_scalar_mul(fxn[:], fxg[:],
                                                IVX[:, m:m + 1])
                    tg = tgp.tile([128, C * IMAGE], bf16, name=f't{g}',
                                  tag=f't{g}')
                    tg_tiles[g] = tg
                    p01 = mm1ps.tile([128, 512], f32, name='p01', tag='p01')
                    p2 = mm1ps.tile([128, IMAGE], f32, name='p2', tag='p2')
                    for c in range(C):
                        dst = p01[:, IMAGE * c:IMAGE * (c + 1)] if c < 2 \
                            else p2[:]
                        nc.tensor.matmul(
                            dst,
                            psall[b][:, 384 * g + 128 * c:
                                     384 * g + 128 * c + 128],
                            fxn[:], start=True, stop=True,
                            skip_group_check=(c == 1))
                    if g % 2 == 0:
                        nc.scalar.mul(tg[:, 0:512], p01[:], IVY[:, m:m + 1])
                        nc.vector.tensor_scalar_mul(tg[:, 512:768], p2[:],
                                                    IVY[:, m:m + 1])
                    else:
                        nc.vector.tensor_scalar_mul(tg[:, 0:512], p01[:],
                                                    IVY[:, m:m + 1])
                        nc.scalar.mul(tg[:, 512:768], p2[:], IVY[:, m:m + 1])
                    if g >= 2:
                        mm2_step(g - 2)

                mm2_step(NG - 2)
                mm2_step(NG - 1)
                for yt in range(2):
                    obA = obp.tile([128, 512], f32, name=f'obA{yt}',
                                   tag=f'obA{yt}')
                    nc.vector.tensor_scalar_mul(obA[:], accs[(yt, 'A')][:],
                                                1.0 / N)
                    nc.sync.dma_start(
                        AP(y_out, ((b * 3) * 256 + 128 * yt) * 256,
                           [[256, 128], [65536, 2], [1, 256]]),
                        obA[:])
                    obB = obp.tile([128, IMAGE], f32, name=f'obB{yt}',
                                   tag=f'obB{yt}')
                    nc.scalar.mul(obB[:], accs[(yt, 'B')][:], 1.0 / N)
                    nc.gpsimd.dma_start(
                        y_out[b, 2, 128 * yt:128 * yt + 128, :], obB[:])

    _split_multi_waits(nc)
    _PROGRAM = nc
    return nc


def _make_in_maps(brushes: np.ndarray, patches: np.ndarray):
    import ml_dtypes
    brushes = np.asarray(brushes, dtype=np.float32)
    patches = np.asarray(patches, dtype=np.float32)
    r = np.arange(128)
    nu = 4 * ((r % 32) // 2) + (r // 32)
    in_maps = []
    for k in range(NCORES):
        bsl = brushes[BLOC * k: BLOC * (k + 1)]        # [2, 64, 2]
        g2w = np.zeros((5, 133), dtype=np.float32)
        g2w[0:4, 0:128] = np.stack([
            bsl[0, nu, 0], bsl[1, nu, 0], bsl[0, nu, 1], bsl[1, nu, 1]])
        g2w[4, 0:128] = (r % 2).astype(np.float32)
        g2w[:, 128:133] = np.eye(5, dtype=np.float32)
        psl = patches[BLOC * k: BLOC * (k + 1)]         # [2, 64, 3, 32, 32]
        pr = psl.reshape(BLOC, NG, 4, C, PH, PW)[..., ::-1, ::-1]
        prt = pr.transpose(0, 2, 5, 1, 3, 4)            # [b, j, q', g, c, p']
        pt2 = np.zeros((BLOC, 4, PW, NG, C, 4, PH), dtype=np.float32)
        for j in range(4):
            pt2[:, j, :, :, :, j, :] = prt[:, j]
        pt2 = pt2.reshape(BLOC, 128, NG * C * 128).astype(ml_dtypes.bfloat16)
        in_maps.append({'g2w': g2w, 'pt2': pt2})
    return in_maps


def kernel(brushes: np.ndarray, patches: np.ndarray) -> np.ndarray:
    from concourse.bass_utils import run_bass_kernel_spmd

    nc = _build_program()
    in_maps = _make_in_maps(brushes, patches)
    res = run_bass_kernel_spmd(nc, in_maps, list(range(NCORES)))
    out = np.concatenate([res.results[k]['y_out'] for k in range(NCORES)],
                         axis=0)
    return out


# revision 37
# speedup vs baseline: 1.1174x; 1.1174x over previous
"""BrushStroke splat kernel for 8 trn2 NeuronCores (v3).

out[b,c,y,x] = mean_n sum_{p,q} Fy[b,n,y,p] Fx[b,n,x,q] patches[b,n,c,p,q]
with Fx/Fy separable Gaussian filter banks (sigma=0.1) normalized over a
padded spatial axis.

Per core (2 batches of 64 strokes, batch-parallel across cores):
 - E rows E[r,t] = exp(-(t - c_r)^2 / (2 sigma^2)), t in [0,319), one per
   (stroke, batch) on partition r = 32j + 2g + b, stored bf16 as one
   [128, 638] x||y tile and bounced to DRAM.
 - One-time prologue computes all 64x32 window-sum normalizers per side
   (window = T - prefix - suffix via tensor_tensor_scan), reciprocal,
   then a DVE 32x32 block transpose -> per-partition scales IVX/IVY.
 - Per group one bf16 DMA gather [128,512] provides both Fx and Fy
   shifted windows; Fx normalizer is one tensor_scalar_mul; Fy windows
   feed MM2 raw (its normalizer is folded into the MM1 PSUM drain).
 - MM1 per (g,c): full-array bf16 matmul with block-diagonal lhsT
   (zeros embedded host-side, bf16, DMA'd in column chunks).
 - MM2 per (b,g): 2 LDW x 4 chained matmuls into 4 PSUM banks
   (yt0/yt1 x {512-wide c0c1, 256-wide c2}).
 - f32 warmup matmuls during the prologue keep the PE HAM warm.
"""
import sys, types
import numpy as np

IMAGE = 256
PAD = 16
EPS = 1e-7
SIGMA2 = 2.0 * 0.1 ** 2
B, N, C, PH, PW = 16, 64, 3, 32, 32
NCORES = 8
BLOC = B // NCORES          # 2 batches per core
NG = N // 4                 # 16 groups of 4 strokes
ET = IMAGE + 2 * PAD + PW - 1   # 319: E row length
WLEN = IMAGE + 2 * PAD          # 288: padded-axis (normalizer window) length


def _install_patches():
    if 'antenv.axon_hooks' not in sys.modules:
        mod = types.ModuleType('antenv.axon_hooks')
        mod._hook = None
        mod.set_axon_ntff_profile_hook = lambda h: setattr(mod, '_hook', h)
        mod.get_axon_ntff_profile_hook = lambda: mod._hook
        sys.modules['antenv.axon_hooks'] = mod
        try:
            from trn_agent_boot.trn_boot import _ntff_profile_via_ctypes
            hook = _ntff_profile_via_ctypes('/opt/axon/libaxon_pjrt.so')
            if hook is not None:
                mod.set_axon_ntff_profile_hook(hook)
        except Exception:
            pass

    import concourse.tile as tile
    import concourse.bass_utils as bass_utils
    from concourse.vector_clock import ScopedClock

    bass_utils.upload_artifacts = lambda tmpdir: 'local://' + tmpdir

    if getattr(tile.TileContext._drain_and_barrier, '_patched', False):
        return

    def _drain_and_barrier(self, tick_clock, wait_clock):
        nc = self.nc
        drain_inst = nc.sync.drain()
        wait_clock.add_sem_waits(
            drain_inst.ins, ScopedClock({None: tick_clock.global_clock}))
        si = drain_inst.ins.sync_info
        waits = list(si.on_wait or [])
        si.on_wait = []
        for w in waits:
            nop = nc.sync.nop()
            nop.ins.sync_info = type(si)(on_wait=[w], on_update=[])
        nc.all_engine_barrier()
        popped = nc._tile_sem_poison_stack.pop()
        assert popped is self._sem_poison
        nc.clear_and_free_semaphores(list(self.sems.allocated().values()))
        nc.all_engine_barrier()

    _drain_and_barrier._patched = True
    tile.TileContext._drain_and_barrier = _drain_and_barrier


def _split_multi_waits(nc):
    """This walrus accepts at most one sync wait per instruction; hoist
    extras onto same-engine NoOps inserted just before."""
    import bass_rust
    n_new = [0]

    def fresh_nop(engine, wait, si_type):
        n_new[0] += 1
        nop = bass_rust.InstNoOp(name=f'I-waitsplit-{n_new[0]}', ins=[], outs=[])
        nop.engine = engine
        nop.sync_info = si_type(on_wait=[wait], on_update=[])
        return nop

    for fn in nc.m.functions:
        for blk in fn.blocks:
            insts = blk.instructions
            i = 0
            while i < len(insts):
                inst = insts[i]
                si = inst.sync_info
                if si is not None and si.on_wait and len(si.on_wait) > 1:
                    waits = list(si.on_wait)
                    si.on_wait = [waits[-1]]
                    for k, w in enumerate(waits[:-1]):
                        insts.insert(i + k, fresh_nop(inst.engine, w, type(si)))
                    i += len(waits) - 1
                i += 1


_PROGRAM = None


def _build_program():
    global _PROGRAM
    if _PROGRAM is not None:
        return _PROGRAM
    _install_patches()
    import concourse.bass as bass
    import concourse.tile as tile
    from concourse import mybir
    from bass_rust import AP

    f32 = mybir.dt.float32
    bf16 = mybir.dt.bfloat16
    AF = mybir.ActivationFunctionType
    AX = mybir.AxisListType
    ALU = mybir.AluOpType

    nc = bass.Bass('TRN2', target_bir_lowering=False, debug=False,
                   num_devices=NCORES)
    # per-core inputs:
    #  g2w  [5,133]: cols 0:128 rows 0-3 brush coords (x_b0,x_b1,y_b0,y_b1;
    #        col r = stroke nu(r)), row 4 = batch-select r%2;
    #        cols 128:133 = 5x5 identity
    #  pt2  [2,128,6144] bf16: full block-diagonal lhsT (zeros embedded)
    g2w_in = nc.declare_dram_parameter('g2w', [5, 133], f32, isOutput=False)
    pt_in = nc.declare_dram_parameter('pt2', [BLOC, 128, 128 * C * NG], bf16,
                                      isOutput=False)
    y_out = nc.declare_dram_parameter('y_out', [BLOC, C, IMAGE, IMAGE], f32,
                                      isOutput=True)

    E_dram = nc.dram_tensor('E_dram', [128, 2 * ET], bf16)

    with tile.TileContext(nc) as tc:
        with tc.tile_pool(name='glob', bufs=1) as gp, \
             tc.tile_pool(name='fxyp', bufs=2) as fxyp, \
             tc.tile_pool(name='fxnp', bufs=4) as fxnp, \
             tc.tile_pool(name='tgp', bufs=1) as tgp, \
             tc.tile_pool(name='obp', bufs=2) as obp, \
             tc.tile_pool(name='mm1ps', bufs=2, space='PSUM') as mm1ps, \
             tc.tile_pool(name='mm2ps', bufs=1, space='PSUM') as mm2ps:
            # ---- input DMAs ----
            g2w = gp.tile([5, 133], f32)
            nc.sync.dma_start(g2w[:], g2w_in[:])
            g2 = g2w[0:4, 0:128]
            idt = g2w[:, 128:133]

            psall = []
            for b in range(BLOC):
                ps = gp.tile([128, 128 * C * NG], bf16, name=f'psall{b}')
                psall.append(ps)
            CHUNK = 1536
            for ch in range(4):          # batch-0 fills, early, on sync
                nc.sync.dma_start(psall[0][:, CHUNK * ch:CHUNK * (ch + 1)],
                                  pt_in[0, :, CHUNK * ch:CHUNK * (ch + 1)])

            # ---- iotas (gpsimd) ----
            it = gp.tile([128, ET], f32)
            nc.gpsimd.iota(it[:], pattern=[[1, ET]], base=0,
                           channel_multiplier=0,
                           allow_small_or_imprecise_dtypes=True)
            # t^2 early (also the warmup matmul operand)
            t2 = gp.tile([128, ET], f32)
            nc.vector.tensor_mul(t2[:], it[:], it[:])

            # ---- brush normalization -> bias vectors ----
            g25 = g2w[0:5, 0:128]
            mn = gp.tile([5, 1], f32)
            mx = gp.tile([5, 1], f32)
            nc.vector.tensor_reduce(mn[:], g25, axis=AX.X, op=ALU.min)
            nc.vector.reduce_max(mx[:], g25, axis=AX.X)
            rng = gp.tile([5, 1], f32)
            nc.vector.tensor_scalar(rng[:], mx[:], mn[:], EPS,
                                    ALU.subtract, ALU.add)
            inv = gp.tile([5, 1], f32)
            nc.vector.reciprocal(inv[:], rng[:])
            gn = gp.tile([5, 128], f32)
            nc.vector.tensor_scalar(gn[:], g25, mn[:], inv[:],
                                    ALU.subtract, ALU.mult)

            tp_ps = mm2ps.tile([128, 5], f32, tag='A0')
            nc.tensor.transpose(tp_ps[:], gn[:], idt)
            tp = gp.tile([128, 5], f32)
            nc.scalar.copy(tp[:], tp_ps[:])
            bs = tp[:, 4:5]

            # bias_x[r] = -(256*gxn[b(r),nu(r)] + CX)
            CXC = PW / 2 - 0.5 + PAD      # 31.5
            CYC = PW / 2 - 0.4 + PAD      # 31.6
            bias = {}
            for nmo, (c0, c1, CC) in {'x': (0, 1, CXC),
                                      'y': (2, 3, CYC)}.items():
                d01 = gp.tile([128, 1], f32, name=f'd{nmo}')
                v = gp.tile([128, 1], f32, name=f'v{nmo}')
                bi = gp.tile([128, 1], f32, name=f'bias{nmo}')
                nc.vector.tensor_sub(d01[:], tp[:, c1:c1 + 1], tp[:, c0:c0 + 1])
                nc.vector.scalar_tensor_tensor(v[:], d01[:], bs,
                                               tp[:, c0:c0 + 1],
                                               ALU.mult, ALU.add)
                nc.vector.tensor_scalar(bi[:], v[:], -float(IMAGE), CC,
                                        ALU.mult, ALU.subtract)
                bias[nmo] = bi

            # ---- E rows: exp(-(t+b)^2/S2); (t+b) first to avoid fp32
            # cancellation in the expanded square
            E = gp.tile([128, 2 * ET], bf16)
            for nmo, off in (('x', 0), ('y', ET)):
                dd = gp.tile([128, ET], f32, name=f'dd{nmo}')
                nc.vector.tensor_scalar_add(dd[:], it[:], bias[nmo][:])
                sq = gp.tile([128, ET], f32, name=f'sq{nmo}')
                nc.vector.tensor_mul(sq[:], dd[:], dd[:])
                nc.scalar.activation(E[:, off:off + ET], sq[:], AF.Exp,
                                     bias=0.0, scale=-1.0 / SIGMA2)
            nc.sync.dma_start(E_dram[:], E[:])

            # ---- warmup matmuls (keep PE busy through the prologue) ----
            for w in range(8):
                wps = mm1ps.tile([128, 512], f32, name='wps', tag='p01')
                nc.tensor.matmul(wps[:, 0:IMAGE], t2[:, 0:128], t2[:, 0:IMAGE],
                                 start=True, stop=True)

            # ---- normalizers: D[r,q'] = T - prefix - suffix; IV = 1/D ----
            def normalizer(off, name):
                Es = E[:, off:off + ET]
                T = gp.tile([128, 1], f32, name=f'T{name}')
                nc.vector.reduce_sum(T[:], Es, axis=AX.X)
                P1 = gp.tile([128, PW], f32, name=f'P1{name}')
                nc.vector.tensor_tensor_scan(P1[:], Es[:, 0:PW], Es[:, 0:PW],
                                             0.0, ALU.add, ALU.bypass)
                P2 = gp.tile([128, PW - 1], f32, name=f'P2{name}')
                nc.vector.tensor_tensor_scan(P2[:], Es[:, WLEN:ET],
                                             Es[:, WLEN:ET],
                                             0.0, ALU.add, ALU.bypass)
                T2 = gp.tile([128, 1], f32, name=f'T2{name}')
                nc.vector.scalar_tensor_tensor(T2[:], T[:], EPS,
                                               P2[:, PW - 2:PW - 1],
                                               ALU.add, ALU.subtract)
                Q = gp.tile([128, PW - 1], f32, name=f'Q{name}')
                nc.vector.tensor_sub(Q[:], P2[:], P1[:, 0:PW - 1])
                D = gp.tile([128, PW], f32, name=f'D{name}')
                nc.vector.tensor_copy(D[:, 0:1], T2[:])
                nc.vector.tensor_scalar_add(D[:, 1:PW], Q[:], T2[:])
                REC = gp.tile([128, PW], f32, name=f'REC{name}')
                nc.vector.reciprocal(REC[:], D[:])
                IV = gp.tile([128, PW], f32, name=f'IV{name}')
                nc.vector.transpose(IV[:], REC[:])
                return IV

            IVX = normalizer(0, 'x')
            IVY = normalizer(ET, 'y')

            # batch-1 lhsT fills (scalar queue, after the E activations)
            for ch in range(4):
                nc.scalar.dma_start(psall[1][:, CHUNK * ch:CHUNK * (ch + 1)],
                                    pt_in[1, :, CHUNK * ch:CHUNK * (ch + 1)])

            # ---- main loops ----
            for b in range(BLOC):
                fxy_tiles = {}
                tg_tiles = {}
                accs = {}
                for yt in range(2):
                    accs[(yt, 'A')] = mm2ps.tile([128, 512], f32,
                                                 name=f'A{yt}', tag=f'A{yt}')
                    accs[(yt, 'B')] = mm2ps.tile([128, IMAGE], f32,
                                                 name=f'B{yt}', tag=f'B{yt}')

                def mm2_step(g):
                    st, sp = (g == 0), (g == NG - 1)
                    tgs = tg_tiles[g]
                    for yt in range(2):
                        l = fxy_tiles[g][:, 128 * yt:128 * yt + 128]
                        nc.tensor.matmul(accs[(yt, 'A')][:], l, tgs[:, 0:512],
                                         start=st, stop=sp)
                        nc.tensor.matmul(accs[(yt, 'B')][:], l,
                                         tgs[:, 512:768], start=st, stop=sp)

                for g in range(NG):
                    m = 2 * g + b
                    fxg = fxnp.tile([128, IMAGE], bf16, name='fxg', tag='fxg')
                    xeng = nc.sync if g % 2 == 0 else nc.scalar
                    xeng.dma_start(
                        fxg[:], AP(E_dram, m * 2 * ET + PAD,
                                   [[2 * ET * 32, 4], [1, PW], [1, IMAGE]]))
                    fyg = fxyp.tile([128, IMAGE], bf16, name=f'fy{g}',
                                    tag=f'fy{g}')
                    nc.gpsimd.dma_start(
                        fyg[:], AP(E_dram, m * 2 * ET + ET + PAD,
                                   [[2 * ET * 32, 4], [1, PW], [1, IMAGE]]))
                    fxy_tiles[g] = fyg
                    fxn = fxnp.tile([128, IMAGE], bf16, name='fxn', tag='fxn')
                    nc.vector.tensor_scalar_mul(fxn[:], fxg[:],
                                                IVX[:, m:m + 1])
                    tg = tgp.tile([128, C * IMAGE], bf16, name=f't{g}',
                                  tag=f't{g}')
                    tg_tiles[g] = tg
                    p01 = mm1ps.tile([128, 512], f32, name='p01', tag='p01')
                    p2 = mm1ps.tile([128, IMAGE], f32, name='p2', tag='p2')
                    for c in range(C):
                        dst = p01[:, IMAGE * c:IMAGE * (c + 1)] if c < 2 \
                            else p2[:]
                        nc.tensor.matmul(
                            dst,
                            psall[b][:, 384 * g + 128 * c:
                                     384 * g + 128 * c + 128],
                            fxn[:], start=True, stop=True,
                            skip_group_check=(c == 1))
                    if g % 2 == 0:
                        nc.scalar.mul(tg[:, 0:512], p01[:], IVY[:, m:m + 1])
                        nc.vector.tensor_scalar_mul(tg[:, 512:768], p2[:],
                                                    IVY[:, m:m + 1])
                    else:
                        nc.vector.tensor_scalar_mul(tg[:, 0:512], p01[:],
                                                    IVY[:, m:m + 1])
                        nc.scalar.mul(tg[:, 512:768], p2[:], IVY[:, m:m + 1])
                    if g >= 2:
                        mm2_step(g - 2)

                mm2_step(NG - 2)
                mm2_step(NG - 1)
                for yt in range(2):
                    obA = obp.tile([128, 512], f32, name=f'obA{yt}',
                                   tag=f'obA{yt}')
                    nc.vector.tensor_scalar_mul(obA[:], accs[(yt, 'A')][:],
                                                1.0 / N)
                    nc.sync.dma_start(
                        AP(y_out, ((b * 3) * 256 + 128 * yt) * 256,
                           [[256, 128], [65536, 2], [1, 256]]),
                        obA[:])
                    obB = obp.tile([128, IMAGE], f32, name=f'obB{yt}',
                                   tag=f'obB{yt}')
                    nc.scalar.mul(obB[:], accs[(yt, 'B')][:], 1.0 / N)
                    nc.gpsimd.dma_start(
                        y_out[b, 2, 128 * yt:128 * yt + 128, :], obB[:])

    _split_multi_waits(nc)
    _PROGRAM = nc
    return nc


def _make_in_maps(brushes: np.ndarray, patches: np.ndarray):
    import ml_dtypes
    brushes = np.asarray(brushes, dtype=np.float32)
    patches = np.asarray(patches, dtype=np.float32)
    r = np.arange(128)
    nu = 4 * ((r % 32) // 2) + (r // 32)
    in_maps = []
    for k in range(NCORES):
        bsl = brushes[BLOC * k: BLOC * (k + 1)]        # [2, 64, 2]
        g2w = np.zeros((5, 133), dtype=np.float32)
        g2w[0:4, 0:128] = np.stack([
            bsl[0, nu, 0], bsl[1, nu, 0], bsl[0, nu, 1], bsl[1, nu, 1]])
        g2w[4, 0:128] = (r % 2).astype(np.float32)
        g2w[:, 128:133] = np.eye(5, dtype=np.float32)
        psl = patches[BLOC * k: BLOC * (k + 1)]         # [2, 64, 3, 32, 32]
        pr = psl.reshape(BLOC, NG, 4, C, PH, PW)[..., ::-1, ::-1]
        prt = pr.transpose(0, 2, 5, 1, 3, 4)            # [b, j, q', g, c, p']
        pt2 = np.zeros((BLOC, 4, PW, NG, C, 4, PH), dtype=np.float32)
        for j in range(4):
            pt2[:, j, :, :, :, j, :] = prt[:, j]
        pt2 = pt2.reshape(BLOC, 128, NG * C * 128).astype(ml_dtypes.bfloat16)
        in_maps.append({'g2w': g2w, 'pt2': pt2})
    return in_maps


def kernel(brushes: np.ndarray, patches: np.ndarray) -> np.ndarray:
    from concourse.bass_utils import run_bass_kernel_spmd

    nc = _build_program()
    in_maps = _make_in_maps(brushes, patches)
    res = run_bass_kernel_spmd(nc, in_maps, list(range(NCORES)))
    out = np.concatenate([res.results[k]['y_out'] for k in range(NCORES)],
                         axis=0)
    return out
